# revision 55
# baseline (speedup 1.0000x reference)
"""MoE-GPT forward on 8 Trainium2 NeuronCores  (~579 us, 5.2x over the
3017 us staged baseline; rel err 6.6e-3 vs the fp32 reference).

Sharding:
- Residual stream replicated on all cores (fp32 in SBUF).
- Attention head-pair sharded: core c (c<6) computes q/k/v, scores,
  softmax and AV for heads (2c, 2c+1) only -- its weight INPUTS carry
  just that head pair, so the SPMD program stays identical across
  cores; an AllGather (rank order = head order) reassembles the full
  attention output; cores 6-7 compute ignored duplicates.  Layer 0
  gathers per q-half so the first AG hides behind the second half's
  compute; layer 1 computes only the last 32 queries (the logits read
  a single position, and MoE/LN are pointwise).
- MoE expert-parallel (core c owns expert c, dense over tokens),
  combined with bf16 AllReduces split in two token halves pipelined
  behind FFN compute; layer 1's MoE runs on the last 32 tokens only.
- lm_head vocab-sharded (8 x 6284 columns), concatenated on host;
  its 9.7 MB weight shard prefetches during layer-1 attention.

Perf notes: all large matmuls run in bf16 (same PE rate as f32r at
free-dim>=256, half the DMA/SBUF bytes); weights are pre-laid-out on
host in partition-major bf16 panels and loaded once per layer with
large DMAs (streaming [128,128] tiles through GpSimd SWDGE serialized
the PE behind ~600ns/descriptor trigger costs); softmax normalization
evacuates PSUM with one copy and defers recip/broadcast/multiply off
the accumulator-release path; causal masking touches only the
diagonal 128-col chunk of each score block.  Gating stays exact fp32
so top-2 routing matches the reference.
"""

import json
from contextlib import ExitStack
import numpy as np
import ml_dtypes

import concourse.bass as bass
import concourse.mybir as mybir
import concourse.tile as tile
from concourse.bass_utils import run_bass_kernel_spmd
from concourse.masks import make_identity

AF = mybir.ActivationFunctionType
ALU = mybir.AluOpType
F32 = mybir.dt.float32
F32R = mybir.dt.float32r
BF16 = mybir.dt.bfloat16
I32 = mybir.dt.int32

L, C, H, E, K, V, T = 2, 768, 12, 8, 2, 50257, 1024
HD = C // H          # 64
F = 4 * C            # 3072
N_CORES = 8
VS = 6284            # vocab shard per core (8*6284 = 50272 >= 50257)
CC = C // 128        # 6 c-chunks
TB = T // 128        # 8 token blocks
FB = F // 128        # 24 f blocks
NEG = -1.0e30
BF = ml_dtypes.bfloat16


def _legalize_bir_json(bir_bytes):
    """This walrus build accepts at most ONE sync wait per instruction;
    split extras onto standalone NoOps on the same engine."""
    m = json.loads(bir_bytes)
    for f in m["functions"]:
        for bb in f["blocks"]:
            out = []
            for inst in bb["instructions"]:
                si = inst.get("sync_info")
                if si:
                    waits = si.get("on_wait") or []
                    if len(waits) > 1:
                        imm = [w for w in waits if w.get("wait_reg") is None]
                        reg = [w for w in waits if w.get("wait_reg") is not None]
                        keep = reg if reg else [imm[-1]]
                        move = imm if reg else imm[:-1]
                        for j, w in enumerate(move):
                            out.append({
                                "debug": inst.get("debug", 0),
                                "engine": inst["engine"],
                                "ins": [], "outs": [],
                                "name": f"{inst['name']}-lw{j}",
                                "opcode": "NoOp",
                                "sync_info": {"on_wait": [w], "on_update": []},
                            })
                        si["on_wait"] = keep
                out.append(inst)
            bb["instructions"] = out
    return json.dumps(m).encode()


def _ln_apply(nc, pool, out_ap, in_ap, g_ap, eps_tile, rows=128):
    """LayerNorm rows of in_ap [rows, C] -> out_ap, gamma g_ap [rows, C]."""
    stats = pool.tile([128, 3, 6], F32, tag="ln_stats")
    mv = pool.tile([128, 2], F32, tag="ln_mv")
    xg = in_ap.rearrange("p (a b) -> p a b", b=256)
    for sg in range(3):
        nc.vector.bn_stats(out=stats[:rows, sg, :], in_=xg[:, sg, :])
    nc.vector.bn_aggr(out=mv[:rows, :], in_=stats[:rows, :, :])
    mean = mv[:rows, 0:1]
    rstd = pool.tile([128, 1], F32, tag="ln_rstd")
    nc.scalar.activation(out=rstd[:rows, :], in_=mv[:rows, 1:2],
                         func=AF.Sqrt, bias=eps_tile[:rows, :])
    nc.vector.reciprocal(out=rstd[:rows, :], in_=rstd[:rows, :])
    tmp = pool.tile([128, C], F32, tag="ln_tmp")
    nc.vector.tensor_scalar(out=tmp[:rows, :], in0=in_ap,
                            scalar1=mean, scalar2=rstd[:rows, :],
                            op0=ALU.subtract, op1=ALU.mult)
    nc.vector.tensor_tensor(out=out_ap, in0=tmp[:rows, :], in1=g_ap,
                            op=ALU.mult)


def build_program():
    nc = bass.Bass()
    # bf16/f32r tiles are deliberate (PE rate); silence the guard
    nc._allow_low_precision_reason = "bf16 matmul inputs are intentional"

    # ---- DRAM parameters (host pre-laid-out, partition-major) ----
    idx = nc.declare_dram_parameter("idx", [1, T], I32, isOutput=False)
    wte = nc.declare_dram_parameter("wte", [V, C], F32, isOutput=False)
    wpe = nc.declare_dram_parameter("wpe", [T, C], F32, isOutput=False)
    ln1_g = nc.declare_dram_parameter("ln1_g", [L, 128, C], F32, isOutput=False)
    ln2_g = nc.declare_dram_parameter("ln2_g", [L, 128, C], F32, isOutput=False)
    lnf_g = nc.declare_dram_parameter("lnf_g", [1, C], F32, isOutput=False)
    evec = nc.declare_dram_parameter("evec", [128, E], F32, isOutput=False)
    gate_wT = nc.declare_dram_parameter("gate_wT", [L, C, E], F32, isOutput=False)
    # qk_pre[l, 0/1]: [128c, 6cc*128d] lhsT panels for THIS core's head
    # pair (0 = q columns, 1 = k columns)
    qk_pre = nc.declare_dram_parameter("qk_pre", [L, 2, 128, 768], BF16, isOutput=False)
    # v_pre: [128c, 6cc, 128d] rhs panel for this core's head pair
    v_pre = nc.declare_dram_parameter("v_pre", [L, 128, CC, 128], BF16, isOutput=False)
    proj_pre0 = None  # placeholder to keep diff small
    proj_pre = nc.declare_dram_parameter("proj_pre", [L, 128, CC, C], BF16, isOutput=False)
    # w1_pre[l, fb]: [128c, 6cc*128f] lhsT panels
    w1_pre = nc.declare_dram_parameter("w1_pre", [L, FB, 128, 768], BF16, isOutput=False)
    # w2_pre[l]: [128f, 24fb, 768c] rhs panels (resident per layer)
    w2_pre = nc.declare_dram_parameter("w2_pre", [L, 128, FB, C], BF16, isOutput=False)
    # wteT_pre: [128c, 6cc, VS] rhs panels for the lm_head shard
    wteT_pre = nc.declare_dram_parameter("wteT_pre", [128, CC, VS], BF16, isOutput=False)
    out = nc.declare_dram_parameter("out", [1, VS], F32, isOutput=True)

    with tile.TileContext(nc) as tc:
        with tc.tile_pool(name="const", bufs=1) as const, \
             tc.tile_pool(name="dram", bufs=1, space="DRAM") as dram, \
             tc.tile_pool(name="xp", bufs=1) as xp, \
             tc.tile_pool(name="small", bufs=2) as small, \
             tc.tile_pool(name="ptrans", bufs=2, space="PSUM") as ptrans, \
             tc.tile_pool(name="psc", bufs=2, space="PSUM") as psc, \
             tc.tile_pool(name="pav", bufs=1, space="PSUM") as pav, \
             tc.tile_pool(name="pbig", bufs=2, space="PSUM") as pbig:

            ident = const.tile([128, 128], F32)
            make_identity(nc, ident)
            eps = const.tile([128, 1], F32)
            nc.vector.memset(eps[:], 1e-5)
            evt = const.tile([128, E], F32)
            nc.sync.dma_start(evt[:], evec[:])
            onesh = const.tile([128, H], F32)
            nc.vector.memset(onesh[:], 1.0)
            ones64f = const.tile([1, HD], F32)
            nc.vector.memset(ones64f[:], 1.0)
            ones64 = const.tile([1, HD], F32R)
            nc.scalar.activation(out=ones64[:], in_=ones64f[:], func=AF.Copy)
            # causal masks for the 4 diagonal sub-block offsets:
            # mask[rel][p, qf] = 0 if qf - rel*128 - p >= 0 else -1e30
            # dmask[p, qf] = 0 if qf - p >= 0 else -1e30 (one 128x128 diagonal
            # block; off-diagonal chunks are either fully open or fully zero)
            dmask = const.tile([128, 128], F32)
            nc.vector.memset(dmask[:], 0.0)
            nc.gpsimd.affine_select(
                out=dmask[:], in_=dmask[:],
                pattern=[[1, 128]], base=0,
                channel_multiplier=-1,
                compare_op=ALU.is_ge, fill=NEG)

            # Residual stream, replicated: X[p, tb, c], token = tb*128+p
            X = xp.tile([128, TB, C], F32)

            # AllReduce staging, bf16, split in two token halves
            ar_in = [dram.tile([T // 2, C], BF16, name=f"ar_in{i}")
                     for i in range(2)]
            ar_out = [dram.tile([T // 2, C], BF16, name=f"ar_out{i}",
                                addr_space="Shared")
                      for i in range(2)]
            ar_in_last = dram.tile([32, C], BF16, name="ar_in_last")
            ar_out_last = dram.tile([32, C], BF16, name="ar_out_last",
                                    addr_space="Shared")
            # attention AllGather staging: each core contributes its
            # head-pair's normalized attention output [128 d-rows, cols];
            # rank order stacks them into the full [C, cols] (+2 junk ranks).
            # Layer 0 gathers per q-half so the first AG hides behind the
            # second half's compute; layer 1 only needs the last query.
            ag_in = [dram.tile([128, T // 2], BF16, name=f"ag_in{i}")
                     for i in range(2)]
            ag_out = [dram.tile([N_CORES * 128, T // 2], BF16,
                                name=f"ag_out{i}", addr_space="Shared")
                      for i in range(2)]
            ag_in1 = dram.tile([128, 32], BF16, name="ag_in1")
            ag_out1 = dram.tile([N_CORES * 128, 32], BF16, name="ag_out1",
                                addr_space="Shared")

            for l in range(L):
              with ExitStack() as les:
                if l == L - 1:
                    # prefetch the big tail weights (lm_head shard + last
                    # layer's w2) during layer-1 attention, off the sync queue
                    lwp = les.enter_context(tc.tile_pool(name="lmoeW", bufs=1))
                    wlm = lwp.tile([128, CC, VS], BF16)
                    nc.gpsimd.dma_start(wlm[:], wteT_pre[:])
                    w2rL = lwp.tile([128, FB, C], BF16)
                    nc.gpsimd.dma_start(w2rL[:], w2_pre[l])
                g1 = const.tile([128, C], F32, tag="g1", bufs=1)
                nc.sync.dma_start(g1[:], ln1_g[l])
                g2 = const.tile([128, C], F32, tag="g2", bufs=1)
                nc.sync.dma_start(g2[:], ln2_g[l])

                # ======== attention (head-pair sharded: this core computes
                # scores/av/softmax for its 2 heads only; layer 1 further
                # restricts the query side to the last token) ========
                with tc.tile_pool(name=f"attn{l}", bufs=1) as ap:
                    qw = T if l == 0 else 32             # query columns
                    qT = ap.tile([128, qw], BF16)        # my 2 heads, x1/8
                    kT = ap.tile([128, T], BF16)
                    vplus = ap.tile([128, TB, 2, HD + 1], BF16)
                    attO = ap.tile([128, qw], BF16)      # my heads' output

                    with tc.tile_pool(name=f"aT{l}", bufs=1) as apT, \
                         tc.tile_pool(name=f"attw{l}", bufs=2) as aw, \
                         tc.tile_pool(name=f"atmpA{l}", bufs=2) as at:
                        aT = apT.tile([128, CC, T], BF16)     # ln1(x)^T

                        def _ln1_one(tb):
                            a = at.tile([128, C], F32, tag="lnout",
                                        name="lnout")
                            _ln_apply(nc, at, a[:], X[:, tb, :], g1[:], eps)
                            for cc in range(CC):
                                pt = ptrans.tile([128, 128], F32, tag="pt", name="pt")
                                nc.tensor.transpose(
                                    out=pt[:],
                                    in_=a[:, cc * 128:(cc + 1) * 128],
                                    identity=ident[:])
                                nc.scalar.activation(
                                    out=aT[:, cc, tb * 128:(tb + 1) * 128],
                                    in_=pt[:], func=AF.Copy)

                        # ln1 + transpose -> aT (bf16); layer 0 interleaves
                        # the embedding; layer 1 interleaves the previous
                        # layer's MoE AllReduce landing
                        if l == 0:
                            with tc.tile_pool(name="embp", bufs=2) as ep:
                                for tb in range(TB):
                                    it = ep.tile([128, 1], I32, tag="idx")
                                    nc.sync.dma_start(
                                        it[:], idx[0:1, tb * 128:(tb + 1) * 128]
                                        .rearrange("a b -> b a"))
                                    emb = ep.tile([128, C], F32, tag="emb")
                                    nc.gpsimd.indirect_dma_start(
                                        out=emb[:], out_offset=None, in_=wte[:, :],
                                        in_offset=bass.IndirectOffsetOnAxis(
                                            ap=it[:, :1], axis=0))
                                    pe = ep.tile([128, C], F32, tag="pe")
                                    nc.sync.dma_start(
                                        pe[:], wpe[tb * 128:(tb + 1) * 128, :])
                                    nc.vector.tensor_add(out=X[:, tb, :],
                                                         in0=emb[:], in1=pe[:])
                                    _ln1_one(tb)
                        def _v_block(tb, vw):
                            pv = psc.tile([128, 128], F32, tag="ps", name="pv")
                            for cc in range(CC):
                                nc.tensor.matmul(pv[:], aT[:, cc, tb * 128:(tb + 1) * 128],
                                                 vw[:, cc, :],
                                                 start=(cc == 0), stop=(cc == CC - 1))
                            dstv = vplus[:, tb, :, 0:HD]
                            nc.vector.tensor_copy(out=dstv, in_=pv[:].rearrange(
                                "p (a b) -> p a b", b=HD))

                        if l == 0:
                            # qT (scaled 1/8), kT
                            for half, dst, scl in ((0, qT, 0.125), (1, kT, 1.0)):
                                wt_ = aw.tile([128, 768], BF16, tag="wqk")
                                nc.sync.dma_start(wt_[:], qk_pre[l, half])
                                for tch in range(2):
                                    ps = psc.tile([128, 512], F32, tag="ps")
                                    for cc in range(CC):
                                        nc.tensor.matmul(ps[:], wt_[:, cc * 128:(cc + 1) * 128],
                                                         aT[:, cc, tch * 512:(tch + 1) * 512],
                                                         start=(cc == 0), stop=(cc == CC - 1))
                                    nc.scalar.activation(
                                        out=dst[:, tch * 512:(tch + 1) * 512],
                                        in_=ps[:], func=AF.Copy, scale=scl)

                            vw = aw.tile([128, CC, 128], BF16, tag="wv", bufs=1)
                            nc.sync.dma_start(vw[:], v_pre[l])
                            for tb in range(TB):
                                nc.scalar.activation(out=vplus[:, tb, :, HD],
                                                     in_=onesh[:, 0:2], func=AF.Copy)
                            for tb in range(TB):
                                _v_block(tb, vw)
                        else:
                            # interleave the previous layer's MoE AllReduce
                            # landing with ln1 + the k/v work for each half,
                            # so AR1 hides behind half-0 compute
                            wtk = aw.tile([128, 768], BF16, tag="wqk")
                            nc.sync.dma_start(wtk[:], qk_pre[l, 1])
                            vw = aw.tile([128, CC, 128], BF16, tag="wv", bufs=1)
                            nc.sync.dma_start(vw[:], v_pre[l])
                            for tb in range(TB):
                                nc.scalar.activation(out=vplus[:, tb, :, HD],
                                                     in_=onesh[:, 0:2], func=AF.Copy)
                            for tcH in range(2):
                                for tloc in range(4):
                                    tb = tcH * 4 + tloc
                                    mo = small.tile([128, C], BF16, tag="mo")
                                    nc.scalar.dma_start(
                                        mo[:],
                                        ar_out[tcH][tloc * 128:(tloc + 1) * 128, :])
                                    nc.vector.tensor_add(out=X[:, tb, :],
                                                         in0=X[:, tb, :], in1=mo[:])
                                for tb in range(tcH * 4, tcH * 4 + 4):
                                    _ln1_one(tb)
                                ps = psc.tile([128, 512], F32, tag="ps", name="psk")
                                for cc in range(CC):
                                    nc.tensor.matmul(
                                        ps[:], wtk[:, cc * 128:(cc + 1) * 128],
                                        aT[:, cc, tcH * 512:(tcH + 1) * 512],
                                        start=(cc == 0), stop=(cc == CC - 1))
                                nc.scalar.activation(
                                    out=kT[:, tcH * 512:(tcH + 1) * 512],
                                    in_=ps[:], func=AF.Copy)
                                for tb in range(tcH * 4, tcH * 4 + 4):
                                    _v_block(tb, vw)
                            # q for the last 32 tokens
                            wtq = aw.tile([128, 768], BF16, tag="wqk")
                            nc.sync.dma_start(wtq[:], qk_pre[l, 0])
                            ps = psc.tile([128, 512], F32, tag="ps", name="psq")
                            for cc in range(CC):
                                nc.tensor.matmul(ps[:, 0:32],
                                                 wtq[:, cc * 128:(cc + 1) * 128],
                                                 aT[:, cc, T - 32:T],
                                                 start=(cc == 0), stop=(cc == CC - 1))
                            nc.scalar.activation(out=qT[:, 0:32], in_=ps[:, 0:32],
                                                 func=AF.Copy, scale=0.125)

                    with tc.tile_pool(name=f"attB{l}", bufs=1) as bp, \
                         tc.tile_pool(name=f"attwB{l}", bufs=2) as bw, \
                         tc.tile_pool(name=f"atmpB{l}", bufs=2) as bt:
                        attT = bp.tile([128, CC, qw], BF16)
                        pw = bw.tile([128, CC, C], BF16, tag="wproj", bufs=1)
                        nc.sync.dma_start(pw[:], proj_pre[l])

                        if l == 0:
                            # scores^T + exp + av^T per q-half; the two 64-row
                            # score matmuls row-pack in the PE array.  Each
                            # q-half AllGathers as soon as it is normalized.
                            for qc in range(2):
                                nkb = 4 * (qc + 1)
                                pas = [pav.tile([HD + 1, 512], F32, tag=f"pa{i}",
                                                name=f"pa{i}")
                                       for i in range(2)]
                                for kb in range(nkb):
                                    ess = []
                                    for i, hp in enumerate((0, HD)):
                                        ps = psc.tile([128, 512], F32, tag="ps")
                                        nc.tensor.matmul(
                                            ps[:], kT[hp:hp + HD, kb * 128:(kb + 1) * 128],
                                            qT[hp:hp + HD, qc * 512:(qc + 1) * 512],
                                            start=True, stop=True)
                                        es = bt.tile([128, 512], BF16, tag=f"es{i}")
                                        if kb >= 4 * qc:  # partial-causal block
                                            rel = kb - 4 * qc
                                            if rel > 0:   # fully-masked chunks
                                                nc.vector.memset(es[:, :rel * 128], 0.0)
                                            ms = bt.tile([128, 128], F32, tag=f"ms{i}")
                                            nc.vector.tensor_tensor(
                                                out=ms[:],
                                                in0=ps[:, rel * 128:(rel + 1) * 128],
                                                in1=dmask[:], op=ALU.add)
                                            nc.scalar.activation(
                                                out=es[:, rel * 128:(rel + 1) * 128],
                                                in_=ms[:], func=AF.Exp)
                                            if rel < 3:   # fully-open chunks
                                                nc.scalar.activation(
                                                    out=es[:, (rel + 1) * 128:],
                                                    in_=ps[:, (rel + 1) * 128:],
                                                    func=AF.Exp)
                                        else:
                                            nc.scalar.activation(out=es[:], in_=ps[:],
                                                                 func=AF.Exp)
                                        ess.append(es)
                                    for i, hp in enumerate((0, HD)):
                                        nc.tensor.matmul(pas[i][:], vplus[:, kb, i, :],
                                                         ess[i][:],
                                                         start=(kb == 0),
                                                         stop=(kb == nkb - 1))
                                # evacuate PSUM fast, normalize off-path
                                for i, hp in enumerate((0, HD)):
                                    au = bt.tile([HD + 1, 512], F32, tag=f"au{i}")
                                    nc.scalar.activation(out=au[:], in_=pas[i][:, :],
                                                         func=AF.Copy)
                                    rs = bt.tile([1, 512], F32R, tag=f"rs{i}")
                                    with nc.allow_low_precision(reason="f32r bcast"):
                                        nc.vector.reciprocal(out=rs[:],
                                                             in_=au[HD:HD + 1, :])
                                    pb = psc.tile([128, 512], F32, tag="ps")
                                    nc.tensor.matmul(pb[:HD, :], ones64[:], rs[:],
                                                     start=True, stop=True)
                                    nc.vector.tensor_tensor(
                                        out=attO[hp:hp + HD, qc * 512:(qc + 1) * 512],
                                        in0=au[:HD, :], in1=pb[:HD, :],
                                        op=ALU.mult)
                                # AllGather this q-half across head pairs
                                nc.sync.dma_start(
                                    ag_in[qc][:], attO[:, qc * 512:(qc + 1) * 512])
                                nc.gpsimd.collective_compute(
                                    "AllGather", ALU.bypass,
                                    replica_groups=[list(range(N_CORES))],
                                    ins=[ag_in[qc].opt()], outs=[ag_out[qc].opt()])

                            # per q-half: load gathered attT, proj + residual
                            for qc in range(2):
                                nc.scalar.dma_start(
                                    attT[:, :, qc * 512:(qc + 1) * 512],
                                    ag_out[qc][0:C, :].rearrange(
                                        "(a p) t -> p a t", p=128))
                                for tb in range(qc * 4, qc * 4 + 4):
                                    for nch in range(2):
                                        py = pbig.tile([128, 384], F32, tag="pb")
                                        for cc in range(CC):
                                            nc.tensor.matmul(
                                                py[:],
                                                attT[:, cc, tb * 128:(tb + 1) * 128],
                                                pw[:, cc, nch * 384:(nch + 1) * 384],
                                                start=(cc == 0), stop=(cc == CC - 1))
                                        nc.vector.tensor_add(
                                            out=X[:, tb, nch * 384:(nch + 1) * 384],
                                            in0=X[:, tb, nch * 384:(nch + 1) * 384],
                                            in1=py[:])
                        else:
                            # layer 1: last 32 queries only (tokens T-32..T-1
                            # cover the one token the logits read)
                            pas = [pav.tile([HD + 1, 512], F32, tag=f"pa{i}",
                                            name=f"pa{i}")
                                   for i in range(2)]
                            for kb in range(TB):
                                ess = []
                                for i, hp in enumerate((0, HD)):
                                    ps = psc.tile([128, 512], F32, tag="ps")
                                    nc.tensor.matmul(
                                        ps[:, 0:32],
                                        kT[hp:hp + HD, kb * 128:(kb + 1) * 128],
                                        qT[hp:hp + HD, 0:32],
                                        start=True, stop=True)
                                    es = bt.tile([128, 32], BF16, tag=f"es1{i}")
                                    if kb == TB - 1:   # diagonal block
                                        ms = bt.tile([128, 32], F32, tag=f"ms1{i}")
                                        nc.vector.tensor_tensor(
                                            out=ms[:], in0=ps[:, 0:32],
                                            in1=dmask[:, 96:128], op=ALU.add)
                                        nc.scalar.activation(out=es[:], in_=ms[:],
                                                             func=AF.Exp)
                                    else:
                                        nc.scalar.activation(out=es[:], in_=ps[:, 0:32],
                                                             func=AF.Exp)
                                    ess.append(es)
                                for i, hp in enumerate((0, HD)):
                                    nc.tensor.matmul(pas[i][:, 0:32],
                                                     vplus[:, kb, i, :], ess[i][:],
                                                     start=(kb == 0),
                                                     stop=(kb == TB - 1))
                            for i, hp in enumerate((0, HD)):
                                au = bt.tile([HD + 1, 32], F32, tag=f"au1{i}")
                                nc.scalar.activation(out=au[:], in_=pas[i][:, 0:32],
                                                     func=AF.Copy)
                                rs = bt.tile([1, 32], F32R, tag=f"rs1{i}")
                                with nc.allow_low_precision(reason="f32r bcast"):
                                    nc.vector.reciprocal(out=rs[:],
                                                         in_=au[HD:HD + 1, :])
                                pb = psc.tile([128, 512], F32, tag="ps")
                                nc.tensor.matmul(pb[:HD, 0:32], ones64[:], rs[:],
                                                 start=True, stop=True)
                                nc.vector.tensor_tensor(
                                    out=attO[hp:hp + HD, :],
                                    in0=au[:HD, :], in1=pb[:HD, 0:32],
                                    op=ALU.mult)
                            nc.sync.dma_start(ag_in1[:], attO[:])
                            nc.gpsimd.collective_compute(
                                "AllGather", ALU.bypass,
                                replica_groups=[list(range(N_CORES))],
                                ins=[ag_in1.opt()], outs=[ag_out1.opt()])
                            nc.scalar.dma_start(
                                attT[:], ag_out1[0:C, :].rearrange(
                                    "(a p) t -> p a t", p=128))
                            # proj + residual for the last 32 tokens of tb 7
                            for nch in range(2):
                                py = pbig.tile([128, 384], F32, tag="pb")
                                for cc in range(CC):
                                    nc.tensor.matmul(
                                        py[:32, :], attT[:, cc, :],
                                        pw[:, cc, nch * 384:(nch + 1) * 384],
                                        start=(cc == 0), stop=(cc == CC - 1))
                                nc.vector.tensor_add(
                                    out=X[96:128, TB - 1, nch * 384:(nch + 1) * 384],
                                    in0=X[96:128, TB - 1, nch * 384:(nch + 1) * 384],
                                    in1=py[:32, :])

                # ln2 + transpose + exact-fp32 gating for one token block;
                # writes comb[:, tb] and aT2b[:, :, tb*128:(tb+1)*128]
                def _ln2_gate_block(tb, mt, aT2b, comb, gwt):
                    a = mt.tile([128, C], F32, tag="lnout2", name="lnout2")
                    _ln_apply(nc, mt, a[:], X[:, tb, :], g2[:], eps)
                    af = mt.tile([128, CC, 128], F32, tag="a2f", name="a2f")
                    for cc in range(CC):
                        pt = ptrans.tile([128, 128], F32, tag="pt", name="pt2")
                        nc.tensor.transpose(out=pt[:],
                                            in_=a[:, cc * 128:(cc + 1) * 128],
                                            identity=ident[:])
                        nc.scalar.activation(out=af[:, cc, :], in_=pt[:], func=AF.Copy)
                        nc.vector.tensor_copy(
                            out=aT2b[:, cc, :] if aT2b.shape[2] == 128
                            else aT2b[:, cc, tb * 128:(tb + 1) * 128],
                            in_=pt[:])
                    pg = pav.tile([128, E], F32, tag="pa0", name="pg")
                    for cc in range(CC):
                        nc.tensor.matmul(pg[:], af[:, cc, :], gwt[:, cc, :],
                                         start=(cc == 0), stop=(cc == CC - 1))
                    lg = mt.tile([128, E], F32, tag="lg", name="lg")
                    nc.vector.tensor_copy(out=lg[:], in_=pg[:])
                    m8 = mt.tile([128, 8], F32, tag="m8", name="m8")
                    nc.vector.max(out=m8[:], in_=lg[:])
                    nv0 = mt.tile([128, 1], F32, tag="nv0", name="nv0")
                    nc.vector.tensor_scalar_mul(out=nv0[:], in0=m8[:, 0:1],
                                                scalar1=-1.0)
                    el = mt.tile([128, E], F32, tag="el", name="el")
                    nc.scalar.activation(out=el[:], in_=lg[:], func=AF.Exp,
                                         bias=nv0[:])
                    e1 = mt.tile([128, 1], F32, tag="e1", name="e1")
                    nc.scalar.activation(out=e1[:], in_=m8[:, 1:2], func=AF.Exp,
                                         bias=nv0[:])
                    nc.vector.tensor_scalar_add(out=e1[:], in0=e1[:], scalar1=1.0)
                    nc.vector.reciprocal(out=e1[:], in_=e1[:])
                    msk = mt.tile([128, E], F32, tag="msk", name="msk")
                    nc.vector.tensor_scalar(out=msk[:], in0=lg[:],
                                            scalar1=m8[:, 1:2], scalar2=None,
                                            op0=ALU.is_ge)
                    nc.vector.tensor_tensor(out=el[:], in0=el[:], in1=msk[:],
                                            op=ALU.mult)
                    nc.vector.tensor_scalar_mul(out=el[:], in0=el[:], scalar1=e1[:])
                    nc.vector.tensor_tensor(out=el[:], in0=el[:], in1=evt[:],
                                            op=ALU.mult)
                    nc.vector.reduce_sum(out=comb[:], in_=el[:],
                                         axis=mybir.AxisListType.X)

                gwt = const.tile([128, CC, E], F32, tag="gw", bufs=1)
                nc.sync.dma_start(gwt[:],
                                  gate_wT[l].rearrange("(a b) e -> b a e", b=128))

                if l < L - 1:
                    # ==== full MoE (dense, expert-parallel); the X += of the
                    # AllReduce result happens at the next layer's entry ====
                    with tc.tile_pool(name=f"moe{l}", bufs=1) as mp, \
                         tc.tile_pool(name=f"mtmp{l}", bufs=2) as mt:
                        aT2b = mp.tile([128, CC, T], BF16)
                        comb = mp.tile([128, TB], F32)
                        w2r = mp.tile([128, FB, C], BF16)
                        nc.sync.dma_start(w2r[:], w2_pre[l])

                        for tb in range(TB):
                            _ln2_gate_block(tb, mt, aT2b, comb[:, tb:tb + 1], gwt)

                        with tc.tile_pool(name=f"moeh{l}", bufs=1) as hp_, \
                             tc.tile_pool(name=f"moew{l}", bufs=3) as mw:
                            for tcH in range(2):
                                hT = hp_.tile([128, FB, 512], BF16, tag="hT")
                                for fb in range(FB):
                                    w1p = mw.tile([128, 768], BF16, tag="w1p")
                                    nc.sync.dma_start(w1p[:], w1_pre[l, fb])
                                    ph = psc.tile([128, 512], F32, tag="ps")
                                    for cc in range(CC):
                                        nc.tensor.matmul(
                                            ph[:], w1p[:, cc * 128:(cc + 1) * 128],
                                            aT2b[:, cc, tcH * 512:(tcH + 1) * 512],
                                            start=(cc == 0), stop=(cc == CC - 1))
                                    nc.scalar.activation(out=hT[:, fb, :],
                                                         in_=ph[:], func=AF.Gelu)

                                # y = hT^T @ w2, scaled by comb -> ar_in[tcH]
                                for tb in range(tcH * 4, tcH * 4 + 4):
                                    tloc = tb - tcH * 4
                                    for nch in range(2):
                                        pyy = pbig.tile([128, 384], F32, tag="pb")
                                        for fb in range(FB):
                                            nc.tensor.matmul(
                                                pyy[:],
                                                hT[:, fb, tloc * 128:(tloc + 1) * 128],
                                                w2r[:, fb, nch * 384:(nch + 1) * 384],
                                                start=(fb == 0), stop=(fb == FB - 1))
                                        ys = mt.tile([128, 384], BF16, tag="ys")
                                        nc.vector.tensor_scalar_mul(
                                            out=ys[:], in0=pyy[:],
                                            scalar1=comb[:, tb:tb + 1])
                                        nc.sync.dma_start(
                                            ar_in[tcH][tloc * 128:(tloc + 1) * 128,
                                                       nch * 384:(nch + 1) * 384], ys[:])
                                nc.gpsimd.collective_compute(
                                    "AllReduce", ALU.add,
                                    replica_groups=[list(range(N_CORES))],
                                    ins=[ar_in[tcH].opt()], outs=[ar_out[tcH].opt()])
                else:
                    # ==== last layer: only the final token block feeds the
                    # logits (MoE is pointwise), so run MoE for tb=TB-1 only;
                    # prefetch the lm_head shard while it runs ====
                    tb = TB - 1
                    with tc.tile_pool(name="moeL", bufs=1) as mp, \
                         tc.tile_pool(name="mtmpL", bufs=2) as mt, \
                         tc.tile_pool(name="moewL", bufs=3) as mw:
                        aT2b = mp.tile([128, CC, 128], BF16)
                        comb = mp.tile([128, 1], F32)
                        w2r = w2rL

                        _ln2_gate_block(tb, mt, aT2b, comb[:, 0:1], gwt)

                        # FFN for the last 32 tokens only (rows 96-127)
                        hT = mp.tile([128, FB, 32], BF16)
                        for fb in range(FB):
                            w1p = mw.tile([128, 768], BF16, tag="w1p")
                            nc.sync.dma_start(w1p[:], w1_pre[l, fb])
                            ph = psc.tile([128, 512], F32, tag="ps")
                            for cc in range(CC):
                                nc.tensor.matmul(ph[:, :32],
                                                 w1p[:, cc * 128:(cc + 1) * 128],
                                                 aT2b[:, cc, 96:128],
                                                 start=(cc == 0), stop=(cc == CC - 1))
                            nc.scalar.activation(out=hT[:, fb, :],
                                                 in_=ph[:, :32], func=AF.Gelu)
                        for nch in range(2):
                            pyy = pbig.tile([128, 384], F32, tag="pb")
                            for fb in range(FB):
                                nc.tensor.matmul(
                                    pyy[:32, :], hT[:, fb, :],
                                    w2r[:, fb, nch * 384:(nch + 1) * 384],
                                    start=(fb == 0), stop=(fb == FB - 1))
                            ys = mt.tile([32, 384], BF16, tag="ys32")
                            nc.vector.tensor_scalar_mul(out=ys[:], in0=pyy[:32, :],
                                                        scalar1=comb[96:128, 0:1])
                            nc.sync.dma_start(
                                ar_in_last[:, nch * 384:(nch + 1) * 384], ys[:])
                        nc.gpsimd.collective_compute(
                            "AllReduce", ALU.add,
                            replica_groups=[list(range(N_CORES))],
                            ins=[ar_in_last.opt()], outs=[ar_out_last.opt()])
                        mo = small.tile([128, C], BF16, tag="mo")
                        nc.sync.dma_start(mo[96:128, :], ar_out_last[:])
                        nc.vector.tensor_add(out=X[96:128, tb, :],
                                             in0=X[96:128, tb, :],
                                             in1=mo[96:128, :])

                        # ---- final LN (last token) + lm_head shard ----
                        gf = mt.tile([1, C], F32, tag="gf", bufs=1)
                        nc.sync.dma_start(gf[:], lnf_g[:])
                        xrow = mt.tile([1, C], F32, tag="xrow", bufs=1)
                        nc.sync.dma_start(xrow[:], X[127:128, TB - 1, :])
                        xl = mt.tile([1, C], F32, tag="xl", bufs=1)
                        _ln_apply(nc, mt, xl[:1, :], xrow[:1, :], gf[:1, :], eps,
                                  rows=1)
                        xlT = mt.tile([128, CC, 1], BF16, tag="xlT", bufs=1)
                        for cc in range(CC):
                            pt = ptrans.tile([128, 128], F32, tag="pt", name="ptl")
                            nc.tensor.transpose(out=pt[:, 0:1],
                                                in_=xl[0:1, cc * 128:(cc + 1) * 128],
                                                identity=ident[0:1, 0:1])
                            nc.scalar.activation(out=xlT[:, cc, 0:1], in_=pt[:, 0:1],
                                                 func=AF.Copy)
                        nvc = VS // 512 + (1 if VS % 512 else 0)
                        for vc in range(nvc):
                            w = min(512, VS - vc * 512)
                            pl = psc.tile([1, 512], F32, tag="ps", name="pl")
                            for cc in range(CC):
                                nc.tensor.matmul(pl[:, :w], xlT[:, cc, 0:1],
                                                 wlm[:, cc, vc * 512:vc * 512 + w],
                                                 start=(cc == 0), stop=(cc == CC - 1))
                            lc = mt.tile([1, 512], F32, tag="lc")
                            nc.vector.tensor_copy(out=lc[:, :w], in_=pl[:, :w])
                            nc.sync.dma_start(out[0:1, vc * 512:vc * 512 + w],
                                              lc[:, :w])

    orig = nc.to_json_bytes
    nc.to_json_bytes = lambda: _legalize_bir_json(orig())
    return nc


_NC_CACHE = None


def _prep_core_weights(c, qkv_w, proj_w, w1, w2, wte):
    """Host-side bf16 partition-major panel layouts for core c.

    Attention is head-pair sharded: core c < 6 owns heads (2c, 2c+1),
    i.e. d-columns [128c, 128c+128) of q/k/v; cores 6-7 duplicate pair 0
    (their AllGather rows land past C and are never read)."""
    hc = c if c < CC else c - CC
    qk_pre = np.empty((L, 2, 128, 768), BF)
    v_pre = np.empty((L, 128, CC * 128), BF)
    proj_pre = np.empty((L, 128, CC * C), BF)
    w1_pre = np.empty((L, FB, 128, 768), BF)
    w2_pre = np.empty((L, 128, FB * C), BF)
    for l in range(L):
        qkvT = qkv_w[l].T.astype(BF)                      # [C, 3C]
        for half in range(2):                             # q then k
            blk = qkvT[:, half * C + hc * 128:half * C + (hc + 1) * 128]
            qk_pre[l, half] = (blk.reshape(CC, 128, 128)
                               .transpose(1, 0, 2).reshape(128, 768))
        vblk = qkvT[:, 2 * C + hc * 128:2 * C + (hc + 1) * 128]
        v_pre[l] = (vblk.reshape(CC, 128, 128)
                    .transpose(1, 0, 2).reshape(128, CC * 128))
        projT = proj_w[l].T.astype(BF)                    # [C, C]
        proj_pre[l] = (projT.reshape(CC, 128, C)
                       .transpose(1, 0, 2).reshape(128, CC * C))
        w1T = w1[l, c].T.astype(BF)                       # [C, F]
        for fb in range(FB):
            blk = w1T[:, fb * 128:(fb + 1) * 128]
            w1_pre[l, fb] = blk.reshape(CC, 128, 128).transpose(1, 0, 2).reshape(128, 768)
        w2T = w2[l, c].T.astype(BF)                       # [F, C]
        w2_pre[l] = (w2T.reshape(FB, 128, C)
                     .transpose(1, 0, 2).reshape(128, FB * C))
    return (qk_pre,
            v_pre.reshape(L, 128, CC, 128),
            proj_pre.reshape(L, 128, CC, C),
            w1_pre,
            w2_pre.reshape(L, 128, FB, C))


def kernel(**inputs):
    global _NC_CACHE
    idx = np.asarray(inputs["idx"]).astype(np.int32)
    wte = np.ascontiguousarray(np.asarray(inputs["wte"], dtype=np.float32))
    wpe = np.ascontiguousarray(np.asarray(inputs["wpe"], dtype=np.float32))
    ln1_g = np.asarray(inputs["ln1_g"], dtype=np.float32)
    qkv_w = np.asarray(inputs["qkv_w"], dtype=np.float32)
    proj_w = np.asarray(inputs["proj_w"], dtype=np.float32)
    ln2_g = np.asarray(inputs["ln2_g"], dtype=np.float32)
    gate_w = np.asarray(inputs["gate_w"], dtype=np.float32)
    w1 = np.asarray(inputs["w1"], dtype=np.float32)
    w2 = np.asarray(inputs["w2"], dtype=np.float32)
    lnf_g = np.asarray(inputs["lnf_g"], dtype=np.float32)

    gate_wT = np.ascontiguousarray(gate_w.transpose(0, 2, 1))
    ln1_rep = np.ascontiguousarray(np.broadcast_to(ln1_g[:, None, :], (L, 128, C)))
    ln2_rep = np.ascontiguousarray(np.broadcast_to(ln2_g[:, None, :], (L, 128, C)))
    wteT_full = np.zeros((C, N_CORES * VS), BF)
    wteT_full[:, :V] = wte.T.astype(BF)

    if _NC_CACHE is None:
        _NC_CACHE = build_program()
    nc = _NC_CACHE

    in_maps = []
    for c in range(N_CORES):
        ev = np.zeros((128, E), np.float32)
        ev[:, c] = 1.0
        qk_pre, v_pre, proj_pre, w1_pre, w2_pre = _prep_core_weights(
            c, qkv_w, proj_w, w1, w2, wte)
        wteT_c = wteT_full[:, c * VS:(c + 1) * VS]        # [C, VS]
        wteT_pre = np.ascontiguousarray(
            wteT_c.reshape(CC, 128, VS).transpose(1, 0, 2))
        in_maps.append({
            "idx": idx,
            "wte": wte,
            "wpe": wpe,
            "ln1_g": ln1_rep,
            "ln2_g": ln2_rep,
            "lnf_g": lnf_g[None, :],
            "evec": ev,
            "gate_wT": gate_wT,
            "qk_pre": qk_pre,
            "v_pre": v_pre,
            "proj_pre": proj_pre,
            "w1_pre": w1_pre,
            "w2_pre": w2_pre,
            "wteT_pre": wteT_pre,
        })

    res = run_bass_kernel_spmd(nc, in_maps, list(range(N_CORES)))
    kernel.last_result = res
    logits = np.concatenate([res.results[c]["out"][0] for c in range(N_CORES)])
    return logits[:V].reshape(1, 1, V).astype(np.float32)
